# revision 4
# baseline (speedup 1.0000x reference)
"""ClusterNorm1dv2 training-mode forward on 8 trn2 NeuronCores.

Sharding: data-parallel over batch B (2048 rows/core). Per-cluster second
moments S_k and sums are computed on-device (bf16 matmuls, fp32 accum),
all-reduced across the 8 cores, then every core runs the tiny [K,D,D]
LDL^T factorization + unit-triangular inversion (vectorized over the 128
clusters on partitions) and whitens its batch shard with fp32 matmuls.

Cluster grouping uses stride-32 sets {g, g+32, g+64, g+96} so that the
128-column selection x[:, g::32] is a single-strided (legal) matmul/
transpose operand; group-product row/col index t = 4*d + j encodes
(feature d, cluster g+32j).
"""

import numpy as np
import ml_dtypes

import concourse.bacc as bacc
import concourse.mybir as mybir
import concourse.tile as tile
from concourse.bass_utils import run_bass_kernel_spmd

F32 = mybir.dt.float32
BF16 = mybir.dt.bfloat16
ALU = mybir.AluOpType
ACTF = mybir.ActivationFunctionType

N_CORES = 8
B, D, K = 16384, 32, 128
BS = B // N_CORES          # 2048 rows per core
NT = BS // 128             # 16 tiles of [128, 4096]
DK = D * K                 # 4096
P = 128

_CACHE = {}


def _build():
    nc = bacc.Bacc("TRN2", target_bir_lowering=False, debug=False,
                   num_devices=N_CORES)

    xs = nc.dram_tensor("xs", [BS, DK], F32, kind="ExternalInput")
    mu0_in = nc.dram_tensor("mu0_in", [D, K], F32, kind="ExternalInput")
    l0_in = nc.dram_tensor("l0_in", [K, D * D], F32, kind="ExternalInput")
    n0_in = nc.dram_tensor("n0_in", [1], F32, kind="ExternalInput")
    ident_in = nc.dram_tensor("ident_in", [P, P], F32, kind="ExternalInput")
    eye_in = nc.dram_tensor("eye_in", [P, D * D], F32, kind="ExternalInput")
    ones_in = nc.dram_tensor("ones_in", [P, 1], BF16, kind="ExternalInput")
    z_out = nc.dram_tensor("z_out", [BS, DK], F32, kind="ExternalOutput")

    with tile.TileContext(nc) as tc:
        with (
            tc.tile_pool(name="consts", bufs=1) as consts,
            tc.tile_pool(name="small", bufs=1) as small,
            tc.tile_pool(name="xpool", bufs=2) as xpool,
            tc.tile_pool(name="xbpool", bufs=2) as xbpool,
            tc.tile_pool(name="stage", bufs=1) as stagep,
            tc.tile_pool(name="ztile", bufs=2) as zpool,
            tc.tile_pool(name="xct", bufs=3) as xctp,
            tc.tile_pool(name="dram", bufs=1, space="DRAM") as dr,
            tc.tile_pool(name="chol", bufs=1) as chp,
            tc.tile_pool(name="choltmp", bufs=2) as chtmp,
        ):
            # ---------------- constants ----------------
            idt = consts.tile([P, P], F32, tag="idt")
            nc.sync.dma_start(idt[:], ident_in[:])
            eye_k = consts.tile([P, D * D], F32, tag="eye")
            nc.sync.dma_start(eye_k[:], eye_in[:])
            ob = consts.tile([P, 1], BF16, tag="ob")
            nc.sync.dma_start(ob[:], ones_in[:])
            n0sb = consts.tile([P, 1], F32, tag="n0")
            nc.sync.dma_start(n0sb[:], n0_in[:].unsqueeze(0).broadcast_to([P, 1]))

            wblk = consts.tile([P, DK], F32, tag="wblk")
            nc.gpsimd.memset(wblk[:], 0.0)
            wmur = consts.tile([P, DK], F32, tag="wmur")

            # mu0 -> [k, d] via PE transpose
            mu0sb = small.tile([D, K], F32, tag="mu0sb")
            nc.sync.dma_start(mu0sb[:], mu0_in[:])
            mu0t = small.tile([P, D], F32, tag="mu0t")

            # ---------------- G = L0 @ L0^T (per cluster) ----------------
            l0sb = small.tile([P, D * D], F32, tag="l0sb")
            nc.sync.dma_start(l0sb[:], l0_in[:])
            l0t = small.tile([P, D * D], F32, tag="l0t")
            nc.vector.tensor_copy(
                l0t[:].rearrange("p (e d) -> p e d", d=D),
                l0sb[:].rearrange("p (d e) -> p d e", e=D).transpose([0, 2, 1]),
            )
            l0tb = dr.tile([P, D * D], F32, tag="l0tb")
            nc.sync.dma_start(l0tb[:], l0t[:])
            lblk = small.tile([P, DK], F32, tag="lblk")
            nc.gpsimd.memset(lblk[:], 0.0)
            for j in range(4):
                nc.sync.dma_start(
                    lblk[j:P:4, :].rearrange("e (g c) -> e g c", c=P)[
                        :, :, 32 * j : 32 * j + 32
                    ],
                    l0tb[:].rearrange("(jj g) (e d) -> jj e g d", jj=4, d=D)[j],
                )
            g_stage = stagep.tile([P, DK], F32, tag="gstage")
            gb = dr.tile([P, D * D], F32, tag="gb")
            with tc.tile_pool(name="gpsum", bufs=2, space="PSUM") as gps:
                for b_ in range(8):
                    pg = gps.tile([P, 512], F32, tag="gps")
                    for q in range(4):
                        g = 4 * b_ + q
                        nc.tensor.matmul(
                            pg[:, 128 * q : 128 * (q + 1)],
                            lblk[:, 128 * g : 128 * (g + 1)],
                            lblk[:, 128 * g : 128 * (g + 1)],
                            start=True, stop=True,
                        )
                    nc.vector.tensor_copy(
                        g_stage[:, 512 * b_ : 512 * (b_ + 1)], pg[:]
                    )
                # transpose mu0 while PE otherwise idle
                pmu = gps.tile([P, D], F32, tag="gmu")
                nc.tensor.transpose(pmu[:], mu0sb[:], idt[0:D, 0:D])
                nc.vector.tensor_copy(mu0t[:], pmu[:])
            for j in range(4):
                nc.sync.dma_start(
                    gb[:].rearrange("(jj g) (d f) -> jj d g f", jj=4, f=D)[j],
                    g_stage[32 * j : 32 * j + 32, :].rearrange(
                        "d (g c) -> d g c", c=P
                    )[:, :, 32 * j : 32 * j + 32],
                )
            g_k = chp.tile([P, D * D], F32, tag="g_k")
            nc.sync.dma_start(g_k[:], gb[:])

            # ---------------- pass 1: S products + sums ----------------
            with tc.tile_pool(name="spsum", bufs=1, space="PSUM") as sps:
                psb = [
                    sps.tile([P, 512], F32, tag=f"sb{i}", name=f"sb{i}")
                    for i in range(8)
                ]
                # start=True clears has_written for the WHOLE psum bank, so
                # only the first-executed matmul per bank may carry it; all
                # other t=0 matmuls rely on overwrite-where-clear semantics.
                bank_started = [False] * 8

                def _st(bk, t):
                    if t != 0:
                        return False
                    if bank_started[bk]:
                        return False
                    bank_started[bk] = True
                    return True

                for t in range(NT):
                    xt = xpool.tile([P, DK], F32, tag="xt")
                    nc.sync.dma_start(xt[:], xs[:][128 * t : 128 * (t + 1), :])
                    xb = xbpool.tile([P, DK], BF16, tag="xb")
                    nc.vector.tensor_copy(xb[:], xt[:])
                    sp = t == NT - 1
                    for g in range(31):
                        sel = xb[:, g:DK:32]
                        bk, q = g // 4, g % 4
                        nc.tensor.matmul(
                            psb[bk][:, 128 * q : 128 * (q + 1)],
                            sel, sel, start=_st(bk, t), stop=sp,
                            skip_group_check=True,
                        )
                        nc.tensor.matmul(
                            psb[7][:, 416 + g : 417 + g],
                            sel, ob[:], start=_st(7, t), stop=sp,
                            skip_group_check=True,
                        )
                    for j in range(4):
                        k = 31 + 32 * j
                        selc = xb[:, k:DK:128]
                        nc.tensor.matmul(
                            psb[7][32 * j : 32 * (j + 1), 384:416],
                            selc, selc, start=_st(7, t), stop=sp,
                            tile_position=(0, 32 * j),
                            skip_group_check=True,
                        )
                        nc.tensor.matmul(
                            psb[7][32 * j : 32 * (j + 1), 447:448],
                            selc, ob[:], start=_st(7, t), stop=sp,
                            tile_position=(0, 32 * j),
                            skip_group_check=True,
                        )

                # de-interleave products into s_stage (col g*128 + j*32 + e)
                s_stage = stagep.tile([P, DK], F32, tag="gstage")
                for b_ in range(7):
                    nc.vector.tensor_copy(
                        s_stage[:, 512 * b_ : 512 * (b_ + 1)].rearrange(
                            "p (q j e) -> p q j e", q=4, j=4
                        ),
                        psb[b_][:].rearrange("p (q e j) -> p q j e", q=4, e=32),
                    )
                nc.vector.tensor_copy(
                    s_stage[:, 3584:3968].rearrange(
                        "p (q j e) -> p q j e", q=3, j=4
                    ),
                    psb[7][:, 0:384].rearrange("p (q e j) -> p q j e", q=3, e=32),
                )
                nc.scalar.copy(s_stage[:, 3968:4032], psb[7][:, 384:448])

            # gather to DRAM AR buffer: rows 0..127 = S[k, (d,e)],
            # rows 128..131 = sums in [d, k] layout
            ar_in = dr.tile([132, 1024], F32, tag="ar_in")
            ar_out = dr.tile([132, 1024], F32, tag="ar_out", addr_space="Shared")
            for j in range(4):
                nc.sync.dma_start(
                    ar_in[:][0:128, :].rearrange(
                        "(jj g) (d e) -> jj d g e", jj=4, e=D
                    )[j][:, 0:31, :],
                    s_stage[j:P:4, :].rearrange("d (g c) -> d g c", c=P)[
                        :, 0:31, 32 * j : 32 * j + 32
                    ],
                )
                nc.sync.dma_start(
                    ar_in[:][31 + 32 * j, :].rearrange("(d e) -> d e", e=D),
                    s_stage[32 * j : 32 * j + 32, 3968:4000],
                )
                nc.sync.dma_start(
                    ar_in[:][128:132, :].rearrange(
                        "r (b c) -> (r b) c", c=K
                    )[:, 32 * j : 32 * j + 31],
                    s_stage[j:P:4, 4000:4031],
                )
                nc.sync.dma_start(
                    ar_in[:][128:132, :].rearrange(
                        "r (b c) -> (r b) c", c=K
                    )[:, 31 + 32 * j : 32 + 32 * j],
                    s_stage[32 * j : 32 * j + 32, 4031:4032],
                )

            nc.gpsimd.collective_compute(
                "AllReduce", ALU.add,
                replica_groups=[list(range(N_CORES))],
                ins=[ar_in.opt()], outs=[ar_out.opt()],
            )

            s_k = chp.tile([P, D * D], F32, tag="s_k")
            nc.sync.dma_start(s_k[:], ar_out[:][0:128, :])
            sums_dk = small.tile([D, K], F32, tag="sums_dk")
            nc.sync.dma_start(
                sums_dk[:],
                ar_out[:][128:132, :].rearrange("r (b c) -> (r b) c", c=K),
            )
            t_k = small.tile([P, D], F32, tag="t_k")
            with tc.tile_pool(name="tpsum", bufs=1, space="PSUM") as tps:
                ptk = tps.tile([P, D], F32, tag="ptk")
                nc.tensor.transpose(ptk[:], sums_dk[:], idt[0:D, 0:D])
                nc.vector.tensor_copy(t_k[:], ptk[:])

            # ---------------- cov assembly (A = new_cov + I) ----------------
            denom = small.tile([P, 1], F32, tag="denom")
            nc.vector.tensor_scalar_add(denom[:], n0sb[:], float(B))
            invden = small.tile([P, 1], F32, tag="invden")
            nc.vector.reciprocal(invden[:], denom[:])
            xbar = small.tile([P, D], F32, tag="xbar")
            nc.vector.tensor_scalar_mul(xbar[:], t_k[:], 1.0 / B)
            nmu = small.tile([P, D], F32, tag="nmu")
            nc.vector.tensor_scalar_mul(nmu[:], mu0t[:], n0sb[:])
            nc.vector.tensor_add(nmu[:], nmu[:], t_k[:])
            nc.vector.tensor_scalar_mul(nmu[:], nmu[:], invden[:])
            xd = small.tile([P, D], F32, tag="xd")
            nc.vector.tensor_sub(xd[:], xbar[:], mu0t[:])

            a_m = chp.tile([P, D * D], F32, tag="a_m")
            tmpo = chp.tile([P, D * D], F32, tag="tmpo")
            nc.vector.tensor_tensor(
                tmpo[:].rearrange("p (d e) -> p d e", e=D),
                t_k[:].unsqueeze(2).broadcast_to([P, D, D]),
                xbar[:].unsqueeze(1).broadcast_to([P, D, D]),
                ALU.mult,
            )
            nc.vector.tensor_sub(a_m[:], s_k[:], tmpo[:])
            nc.vector.tensor_scalar_mul(a_m[:], a_m[:], invden[:])
            coefg = small.tile([P, 1], F32, tag="coefg")
            nc.vector.tensor_tensor(coefg[:], n0sb[:], invden[:], ALU.mult)
            nc.vector.scalar_tensor_tensor(
                a_m[:], g_k[:], coefg[:], a_m[:], ALU.mult, ALU.add
            )
            coefx = small.tile([P, 1], F32, tag="coefx")
            nc.vector.tensor_scalar_mul(coefx[:], n0sb[:], float(B))
            nc.vector.tensor_tensor(coefx[:], coefx[:], invden[:], ALU.mult)
            nc.vector.tensor_tensor(coefx[:], coefx[:], invden[:], ALU.mult)
            nc.vector.tensor_tensor(
                tmpo[:].rearrange("p (d e) -> p d e", e=D),
                xd[:].unsqueeze(2).broadcast_to([P, D, D]),
                xd[:].unsqueeze(1).broadcast_to([P, D, D]),
                ALU.mult,
            )
            nc.vector.scalar_tensor_tensor(
                a_m[:], tmpo[:], coefx[:], a_m[:], ALU.mult, ALU.add
            )
            nc.vector.tensor_add(a_m[:], a_m[:], eye_k[:])

            # ---------------- LDL^T factorization ----------------
            l_m = chp.tile([P, D * D], F32, tag="l_m")
            av = a_m[:].rearrange("p (i k) -> p i k", k=D)
            for j in range(D - 1):
                n = D - 1 - j
                invd = chtmp.tile([P, 1], F32, tag="invd")
                nc.vector.reciprocal(invd[:], a_m[:, 33 * j : 33 * j + 1])
                rawc = a_m[:, 32 * (j + 1) + j : D * D : 32]
                lcol = l_m[:, 32 * (j + 1) + j : D * D : 32]
                nc.vector.tensor_scalar_mul(lcol, rawc, invd[:])
                tmpu = chtmp.tile([P, 31, 31], F32, tag="tmpu")
                nc.vector.tensor_tensor(
                    tmpu[:, 0:n, 0:n],
                    lcol.unsqueeze(2).broadcast_to([P, n, n]),
                    rawc.unsqueeze(1).broadcast_to([P, n, n]),
                    ALU.mult,
                )
                nc.vector.tensor_sub(
                    av[:, j + 1 : D, j + 1 : D],
                    av[:, j + 1 : D, j + 1 : D],
                    tmpu[:, 0:n, 0:n],
                )

            dvec = small.tile([P, D], F32, tag="dvec")
            nc.vector.tensor_copy(dvec[:], a_m[:, 0 : D * D : 33])
            rsq = small.tile([P, D], F32, tag="rsq")
            nc.vector.reciprocal(rsq[:], dvec[:])
            nc.scalar.activation(rsq[:], rsq[:], ACTF.Sqrt)
            nt1 = small.tile([P, D], F32, tag="nt1")
            for _ in range(2):  # Newton refinement of rsqrt
                nc.vector.tensor_tensor(nt1[:], rsq[:], rsq[:], ALU.mult)
                nc.vector.tensor_tensor(nt1[:], nt1[:], dvec[:], ALU.mult)
                nc.vector.tensor_scalar(
                    out=nt1[:], in0=nt1[:], scalar1=-0.5, scalar2=1.5,
                    op0=ALU.mult, op1=ALU.add,
                )
                nc.vector.tensor_tensor(rsq[:], rsq[:], nt1[:], ALU.mult)

            # ---------------- unit-lower inverse, scale rows ----------------
            wu = chp.tile([P, D * D], F32, tag="wu")
            nc.vector.tensor_copy(wu[:], eye_k[:])
            wv = wu[:].rearrange("p (i c) -> p i c", c=D)
            for jc in range(D - 1):
                n = D - 1 - jc
                lcol = l_m[:, 32 * (jc + 1) + jc : D * D : 32]
                roww = wv[:, jc, 0 : jc + 1]
                tmpu = chtmp.tile([P, 31, 31], F32, tag="tmpu")
                nc.vector.tensor_tensor(
                    tmpu[:, 0:n, 0 : jc + 1],
                    lcol.unsqueeze(2).broadcast_to([P, n, jc + 1]),
                    roww.unsqueeze(1).broadcast_to([P, n, jc + 1]),
                    ALU.mult,
                )
                nc.vector.tensor_sub(
                    wv[:, jc + 1 : D, 0 : jc + 1],
                    wv[:, jc + 1 : D, 0 : jc + 1],
                    tmpu[:, 0:n, 0 : jc + 1],
                )
            nc.vector.tensor_tensor(
                wv, wv, rsq[:].unsqueeze(2).broadcast_to([P, D, D]), ALU.mult
            )

            # W^T (e-major) for the Wblk scatter; -W@new_mu for the bias
            wt = chp.tile([P, D * D], F32, tag="wt")
            nc.vector.tensor_copy(
                wt[:].rearrange("p (e d) -> p e d", d=D),
                wv.transpose([0, 2, 1]),
            )
            tmpw = chp.tile([P, D * D], F32, tag="tmpw")
            nc.vector.tensor_tensor(
                tmpw[:].rearrange("p (d e) -> p d e", e=D),
                wv,
                nmu[:].unsqueeze(1).broadcast_to([P, D, D]),
                ALU.mult,
            )
            wmu = small.tile([P, D], F32, tag="wmu")
            nc.vector.tensor_reduce(
                wmu[:], tmpw[:].rearrange("p (d e) -> p d e", e=D),
                mybir.AxisListType.X, ALU.add,
            )
            nc.vector.tensor_scalar_mul(wmu[:], wmu[:], -1.0)

            wtb = dr.tile([P, D * D], F32, tag="wtb")
            nc.sync.dma_start(wtb[:], wt[:])
            wmub = dr.tile([P, D], F32, tag="wmub")
            nc.sync.dma_start(wmub[:], wmu[:])
            for j in range(4):
                nc.sync.dma_start(
                    wblk[j:P:4, :].rearrange("e (g c) -> e g c", c=P)[
                        :, :, 32 * j : 32 * j + 32
                    ],
                    wtb[:].rearrange("(jj g) (e d) -> jj e g d", jj=4, d=D)[j],
                )
                nc.sync.dma_start(
                    wmur[:].rearrange("p (g c) -> p g c", c=P)[
                        :, :, 32 * j : 32 * j + 32
                    ],
                    wmub[:].rearrange("(jj g) d -> jj g d", jj=4)[j]
                    .unsqueeze(0).broadcast_to([P, 32, D]),
                )

            # ---------------- pass 2: whitening ----------------
            with (
                tc.tile_pool(name="xtp", bufs=2, space="PSUM") as xtps,
                tc.tile_pool(name="zp", bufs=2, space="PSUM") as zps,
            ):
                for t in range(NT):
                    xt2 = xpool.tile([P, DK], F32, tag="xt")
                    nc.sync.dma_start(xt2[:], xs[:][128 * t : 128 * (t + 1), :])
                    zt = zpool.tile([P, DK], F32, tag="zt")
                    for q in range(8):
                        pxt = xtps.tile([P, 512], F32, tag="pxt")
                        for gg in range(4):
                            g = 4 * q + gg
                            nc.tensor.transpose(
                                pxt[:, 128 * gg : 128 * (gg + 1)],
                                xt2[:, g:DK:32], idt[:],
                            )
                        xct = xctp.tile([P, 512], F32, tag="xct")
                        nc.scalar.copy(xct[:], pxt[:])
                        pz = zps.tile([P, 512], F32, tag="pz")
                        for gg in range(4):
                            g = 4 * q + gg
                            nc.tensor.matmul(
                                pz[:, 128 * gg : 128 * (gg + 1)],
                                xct[:, 128 * gg : 128 * (gg + 1)],
                                wblk[:, 128 * g : 128 * (g + 1)],
                                start=True, stop=True,
                            )
                        nc.vector.tensor_tensor(
                            zt[:].rearrange(
                                "p (d jj gb) -> p gb jj d", jj=4, gb=32
                            )[:, 4 * q : 4 * q + 4, :, :],
                            pz[:].rearrange("p (gg j d) -> p gg j d", gg=4, d=D),
                            wmur[:, 512 * q : 512 * (q + 1)].rearrange(
                                "p (gg j d) -> p gg j d", gg=4, d=D
                            ),
                            ALU.add,
                        )
                    nc.sync.dma_start(z_out[:][128 * t : 128 * (t + 1), :], zt[:])

    nc.compile()
    return nc


def _get_nc():
    if "nc" not in _CACHE:
        _CACHE["nc"] = _build()
    return _CACHE["nc"]


def kernel(x, mu_0, L_0, n_0):
    x = np.ascontiguousarray(np.asarray(x, dtype=np.float32))
    mu_0 = np.ascontiguousarray(np.asarray(mu_0, dtype=np.float32))
    L_0 = np.ascontiguousarray(np.asarray(L_0, dtype=np.float32))
    n_0 = np.ascontiguousarray(np.asarray(n_0, dtype=np.float32))

    nc = _get_nc()
    ident = np.eye(P, dtype=np.float32)
    eye_k = np.broadcast_to(
        np.eye(D, dtype=np.float32).reshape(1, D * D), (P, D * D)
    ).copy()
    onesb = np.ones((P, 1), dtype=ml_dtypes.bfloat16)

    x2 = x.reshape(B, DK)
    in_maps = []
    for c in range(N_CORES):
        in_maps.append({
            "xs": x2[c * BS : (c + 1) * BS],
            "mu0_in": mu_0,
            "l0_in": L_0.reshape(K, D * D),
            "n0_in": n_0,
            "ident_in": ident,
            "eye_in": eye_k,
            "ones_in": onesb,
        })
    res = run_bass_kernel_spmd(nc, in_maps, core_ids=list(range(N_CORES)))
    z = np.concatenate(
        [res.results[c]["z_out"] for c in range(N_CORES)], axis=0
    )
    return z.reshape(B, D, K)


# revision 5
# speedup vs baseline: 100.7029x; 100.7029x over previous
"""ClusterNorm1dv2 training-mode forward on 8 trn2 NeuronCores.

Sharding: data-parallel over batch B (2048 rows/core). Per-cluster second
moments S_k and sums are computed on-device (bf16 matmuls, fp32 accum),
all-reduced across the 8 cores, then every core runs the tiny [K,D,D]
LDL^T factorization + unit-triangular inversion (vectorized over the 128
clusters on partitions) and whitens its batch shard with fp32 matmuls.

Cluster grouping uses stride-32 sets {g, g+32, g+64, g+96} so that the
128-column selection x[:, g::32] is a single-strided (legal) matmul/
transpose operand; group-product row/col index t = 4*d + j encodes
(feature d, cluster g+32j).
"""

import numpy as np
import ml_dtypes

import concourse.bacc as bacc
import concourse.mybir as mybir
import concourse.tile as tile
from concourse.bass_utils import run_bass_kernel_spmd

F32 = mybir.dt.float32
BF16 = mybir.dt.bfloat16
ALU = mybir.AluOpType
ACTF = mybir.ActivationFunctionType

N_CORES = 8
B, D, K = 16384, 32, 128
BS = B // N_CORES          # 2048 rows per core
NT = BS // 128             # 16 tiles of [128, 4096]
DK = D * K                 # 4096
P = 128

_CACHE = {}


def _build():
    nc = bacc.Bacc("TRN2", target_bir_lowering=False, debug=False,
                   num_devices=N_CORES)

    xs = nc.dram_tensor("xs", [BS, DK], F32, kind="ExternalInput")
    mu0_in = nc.dram_tensor("mu0_in", [D, K], F32, kind="ExternalInput")
    l0_in = nc.dram_tensor("l0_in", [K, D * D], F32, kind="ExternalInput")
    n0_in = nc.dram_tensor("n0_in", [1], F32, kind="ExternalInput")
    ident_in = nc.dram_tensor("ident_in", [P, P], F32, kind="ExternalInput")
    eye_in = nc.dram_tensor("eye_in", [P, D * D], F32, kind="ExternalInput")
    ones_in = nc.dram_tensor("ones_in", [P, 1], BF16, kind="ExternalInput")
    z_out = nc.dram_tensor("z_out", [BS, DK], F32, kind="ExternalOutput")

    with tile.TileContext(nc) as tc:
        with (
            tc.tile_pool(name="consts", bufs=1) as consts,
            tc.tile_pool(name="small", bufs=1) as small,
            tc.tile_pool(name="xpool", bufs=2) as xpool,
            tc.tile_pool(name="xbpool", bufs=2) as xbpool,
            tc.tile_pool(name="stage", bufs=1) as stagep,
            tc.tile_pool(name="ztile", bufs=2) as zpool,
            tc.tile_pool(name="xct", bufs=3) as xctp,
            tc.tile_pool(name="dram", bufs=1, space="DRAM") as dr,
            tc.tile_pool(name="chol", bufs=1) as chp,
            tc.tile_pool(name="choltmp", bufs=2) as chtmp,
        ):
            # ---------------- constants ----------------
            idt = consts.tile([P, P], F32, tag="idt")
            nc.sync.dma_start(idt[:], ident_in[:])
            eye_k = consts.tile([P, D * D], F32, tag="eye")
            nc.sync.dma_start(eye_k[:], eye_in[:])
            ob = consts.tile([P, 1], BF16, tag="ob")
            nc.sync.dma_start(ob[:], ones_in[:])
            n0sb = consts.tile([P, 1], F32, tag="n0")
            nc.sync.dma_start(n0sb[:], n0_in[:].unsqueeze(0).broadcast_to([P, 1]))

            wblk = consts.tile([P, DK], F32, tag="wblk")
            nc.gpsimd.memset(wblk[:], 0.0)
            wmur = consts.tile([P, DK], F32, tag="wmur")

            # mu0 -> [k, d] via PE transpose
            mu0sb = small.tile([D, K], F32, tag="mu0sb")
            nc.sync.dma_start(mu0sb[:], mu0_in[:])
            mu0t = small.tile([P, D], F32, tag="mu0t")

            # ---------------- G = L0 @ L0^T (per cluster) ----------------
            l0sb = small.tile([P, D * D], F32, tag="l0sb")
            nc.sync.dma_start(l0sb[:], l0_in[:])
            l0t = small.tile([P, D * D], F32, tag="l0t")
            nc.vector.tensor_copy(
                l0t[:].rearrange("p (e d) -> p e d", d=D),
                l0sb[:].rearrange("p (d e) -> p d e", e=D).transpose([0, 2, 1]),
            )
            l0tb = dr.tile([P, D * D], F32, tag="l0tb")
            nc.sync.dma_start(l0tb[:], l0t[:])
            lblk = small.tile([P, DK], F32, tag="lblk")
            nc.gpsimd.memset(lblk[:], 0.0)
            for j in range(4):
                nc.sync.dma_start(
                    lblk[j:P:4, :].rearrange("e (g c) -> e g c", c=P)[
                        :, :, 32 * j : 32 * j + 32
                    ],
                    l0tb[:].rearrange("(jj g) (e d) -> jj e g d", jj=4, d=D)[j],
                )
            g_stage = stagep.tile([P, DK], F32, tag="gstage")
            gb = dr.tile([P, D * D], F32, tag="gb")
            with tc.tile_pool(name="gpsum", bufs=2, space="PSUM") as gps:
                for b_ in range(8):
                    pg = gps.tile([P, 512], F32, tag="gps")
                    for q in range(4):
                        g = 4 * b_ + q
                        nc.tensor.matmul(
                            pg[:, 128 * q : 128 * (q + 1)],
                            lblk[:, 128 * g : 128 * (g + 1)],
                            lblk[:, 128 * g : 128 * (g + 1)],
                            start=True, stop=True,
                        )
                    nc.vector.tensor_copy(
                        g_stage[:, 512 * b_ : 512 * (b_ + 1)], pg[:]
                    )
                # transpose mu0 while PE otherwise idle
                pmu = gps.tile([P, D], F32, tag="gmu")
                nc.tensor.transpose(pmu[:], mu0sb[:], idt[0:D, 0:D])
                nc.vector.tensor_copy(mu0t[:], pmu[:])
            for j in range(4):
                nc.sync.dma_start(
                    gb[:].rearrange("(jj g) (d f) -> jj d g f", jj=4, f=D)[j],
                    g_stage[32 * j : 32 * j + 32, :].rearrange(
                        "d (g c) -> d g c", c=P
                    )[:, :, 32 * j : 32 * j + 32],
                )
            g_k = chp.tile([P, D * D], F32, tag="g_k")
            nc.sync.dma_start(g_k[:], gb[:])

            # ---------------- pass 1: S products + sums ----------------
            with tc.tile_pool(name="spsum", bufs=1, space="PSUM") as sps:
                psb = [
                    sps.tile([P, 512], F32, tag=f"sb{i}", name=f"sb{i}")
                    for i in range(8)
                ]
                # start=True clears has_written for the WHOLE psum bank, so
                # only the first-executed matmul per bank may carry it; all
                # other t=0 matmuls rely on overwrite-where-clear semantics.
                bank_started = [False] * 8

                def _st(bk, t):
                    if t != 0:
                        return False
                    if bank_started[bk]:
                        return False
                    bank_started[bk] = True
                    return True

                for t in range(NT):
                    xt = xpool.tile([P, DK], F32, tag="xt")
                    nc.sync.dma_start(xt[:], xs[:][128 * t : 128 * (t + 1), :])
                    xb = xbpool.tile([P, DK], BF16, tag="xb")
                    nc.vector.tensor_copy(xb[:], xt[:])
                    sp = t == NT - 1
                    for g in range(31):
                        sel = xb[:, g:DK:32]
                        bk, q = g // 4, g % 4
                        nc.tensor.matmul(
                            psb[bk][:, 128 * q : 128 * (q + 1)],
                            sel, sel, start=_st(bk, t), stop=sp,
                            skip_group_check=True,
                        )
                        nc.tensor.matmul(
                            psb[7][:, 416 + g : 417 + g],
                            sel, ob[:], start=_st(7, t), stop=sp,
                            skip_group_check=True,
                        )
                    for j in range(4):
                        k = 31 + 32 * j
                        selc = xb[:, k:DK:128]
                        nc.tensor.matmul(
                            psb[7][32 * j : 32 * (j + 1), 384:416],
                            selc, selc, start=_st(7, t), stop=sp,
                            tile_position=(0, 32 * j),
                            skip_group_check=True,
                        )
                        nc.tensor.matmul(
                            psb[7][32 * j : 32 * (j + 1), 447:448],
                            selc, ob[:], start=_st(7, t), stop=sp,
                            tile_position=(0, 32 * j),
                            skip_group_check=True,
                        )

                # de-interleave products into s_stage (col g*128 + j*32 + e)
                s_stage = stagep.tile([P, DK], F32, tag="gstage")
                for b_ in range(7):
                    nc.vector.tensor_copy(
                        s_stage[:, 512 * b_ : 512 * (b_ + 1)].rearrange(
                            "p (q j e) -> p q j e", q=4, j=4
                        ),
                        psb[b_][:].rearrange("p (q e j) -> p q j e", q=4, e=32),
                    )
                nc.vector.tensor_copy(
                    s_stage[:, 3584:3968].rearrange(
                        "p (q j e) -> p q j e", q=3, j=4
                    ),
                    psb[7][:, 0:384].rearrange("p (q e j) -> p q j e", q=3, e=32),
                )
                nc.scalar.copy(s_stage[:, 3968:4032], psb[7][:, 384:448])

            # gather to DRAM AR buffer: rows 0..127 = S[k, (d,e)],
            # rows 128..131 = sums in [d, k] layout
            ar_in = dr.tile([132, 1024], F32, tag="ar_in")
            ar_out = dr.tile([132, 1024], F32, tag="ar_out", addr_space="Shared")
            for j in range(4):
                nc.sync.dma_start(
                    ar_in[:][0:128, :].rearrange(
                        "(jj g) (d e) -> jj d g e", jj=4, e=D
                    )[j][:, 0:31, :],
                    s_stage[j:P:4, :].rearrange("d (g c) -> d g c", c=P)[
                        :, 0:31, 32 * j : 32 * j + 32
                    ],
                )
                nc.sync.dma_start(
                    ar_in[:][31 + 32 * j, :].rearrange("(d e) -> d e", e=D),
                    s_stage[32 * j : 32 * j + 32, 3968:4000],
                )
                nc.sync.dma_start(
                    ar_in[:][128:132, :].rearrange(
                        "r (b c) -> (r b) c", c=K
                    )[:, 32 * j : 32 * j + 31],
                    s_stage[j:P:4, 4000:4031],
                )
                nc.sync.dma_start(
                    ar_in[:][128:132, :].rearrange(
                        "r (b c) -> (r b) c", c=K
                    )[:, 31 + 32 * j : 32 + 32 * j],
                    s_stage[32 * j : 32 * j + 32, 4031:4032],
                )

            nc.gpsimd.collective_compute(
                "AllReduce", ALU.add,
                replica_groups=[list(range(N_CORES))],
                ins=[ar_in.opt()], outs=[ar_out.opt()],
            )

            s_k = chp.tile([P, D * D], F32, tag="s_k")
            nc.sync.dma_start(s_k[:], ar_out[:][0:128, :])
            sums_dk = small.tile([D, K], F32, tag="sums_dk")
            nc.sync.dma_start(
                sums_dk[:],
                ar_out[:][128:132, :].rearrange("r (b c) -> (r b) c", c=K),
            )
            t_k = small.tile([P, D], F32, tag="t_k")
            with tc.tile_pool(name="tpsum", bufs=1, space="PSUM") as tps:
                ptk = tps.tile([P, D], F32, tag="ptk")
                nc.tensor.transpose(ptk[:], sums_dk[:], idt[0:D, 0:D])
                nc.vector.tensor_copy(t_k[:], ptk[:])

            # ---------------- cov assembly (A = new_cov + I) ----------------
            denom = small.tile([P, 1], F32, tag="denom")
            nc.vector.tensor_scalar_add(denom[:], n0sb[:], float(B))
            invden = small.tile([P, 1], F32, tag="invden")
            nc.vector.reciprocal(invden[:], denom[:])
            xbar = small.tile([P, D], F32, tag="xbar")
            nc.vector.tensor_scalar_mul(xbar[:], t_k[:], 1.0 / B)
            nmu = small.tile([P, D], F32, tag="nmu")
            nc.vector.tensor_scalar_mul(nmu[:], mu0t[:], n0sb[:])
            nc.vector.tensor_add(nmu[:], nmu[:], t_k[:])
            nc.vector.tensor_scalar_mul(nmu[:], nmu[:], invden[:])
            xd = small.tile([P, D], F32, tag="xd")
            nc.vector.tensor_sub(xd[:], xbar[:], mu0t[:])

            a_m = chp.tile([P, D * D], F32, tag="a_m")
            tmpo = chp.tile([P, D * D], F32, tag="tmpo")
            nc.vector.tensor_tensor(
                tmpo[:].rearrange("p (d e) -> p d e", e=D),
                t_k[:].unsqueeze(2).broadcast_to([P, D, D]),
                xbar[:].unsqueeze(1).broadcast_to([P, D, D]),
                ALU.mult,
            )
            nc.vector.tensor_sub(a_m[:], s_k[:], tmpo[:])
            nc.vector.tensor_scalar_mul(a_m[:], a_m[:], invden[:])
            coefg = small.tile([P, 1], F32, tag="coefg")
            nc.vector.tensor_tensor(coefg[:], n0sb[:], invden[:], ALU.mult)
            nc.vector.scalar_tensor_tensor(
                a_m[:], g_k[:], coefg[:], a_m[:], ALU.mult, ALU.add
            )
            coefx = small.tile([P, 1], F32, tag="coefx")
            nc.vector.tensor_scalar_mul(coefx[:], n0sb[:], float(B))
            nc.vector.tensor_tensor(coefx[:], coefx[:], invden[:], ALU.mult)
            nc.vector.tensor_tensor(coefx[:], coefx[:], invden[:], ALU.mult)
            nc.vector.tensor_tensor(
                tmpo[:].rearrange("p (d e) -> p d e", e=D),
                xd[:].unsqueeze(2).broadcast_to([P, D, D]),
                xd[:].unsqueeze(1).broadcast_to([P, D, D]),
                ALU.mult,
            )
            nc.vector.scalar_tensor_tensor(
                a_m[:], tmpo[:], coefx[:], a_m[:], ALU.mult, ALU.add
            )
            nc.vector.tensor_add(a_m[:], a_m[:], eye_k[:])

            # ---------------- LDL^T factorization ----------------
            l_m = chp.tile([P, D * D], F32, tag="l_m")
            av = a_m[:].rearrange("p (i k) -> p i k", k=D)
            for j in range(D - 1):
                n = D - 1 - j
                invd = chtmp.tile([P, 1], F32, tag="invd")
                nc.vector.reciprocal(invd[:], a_m[:, 33 * j : 33 * j + 1])
                rawc = a_m[:, 32 * (j + 1) + j : D * D : 32]
                lcol = l_m[:, 32 * (j + 1) + j : D * D : 32]
                nc.vector.tensor_scalar_mul(lcol, rawc, invd[:])
                tmpu = chtmp.tile([P, 31, 31], F32, tag="tmpu")
                nc.vector.tensor_tensor(
                    tmpu[:, 0:n, 0:n],
                    lcol.unsqueeze(2).broadcast_to([P, n, n]),
                    rawc.unsqueeze(1).broadcast_to([P, n, n]),
                    ALU.mult,
                )
                nc.vector.tensor_sub(
                    av[:, j + 1 : D, j + 1 : D],
                    av[:, j + 1 : D, j + 1 : D],
                    tmpu[:, 0:n, 0:n],
                )

            dvec = small.tile([P, D], F32, tag="dvec")
            nc.vector.tensor_copy(dvec[:], a_m[:, 0 : D * D : 33])
            rsq = small.tile([P, D], F32, tag="rsq")
            nc.vector.reciprocal(rsq[:], dvec[:])
            nc.scalar.activation(rsq[:], rsq[:], ACTF.Sqrt)
            nt1 = small.tile([P, D], F32, tag="nt1")
            for _ in range(2):  # Newton refinement of rsqrt
                nc.vector.tensor_tensor(nt1[:], rsq[:], rsq[:], ALU.mult)
                nc.vector.tensor_tensor(nt1[:], nt1[:], dvec[:], ALU.mult)
                nc.vector.tensor_scalar(
                    out=nt1[:], in0=nt1[:], scalar1=-0.5, scalar2=1.5,
                    op0=ALU.mult, op1=ALU.add,
                )
                nc.vector.tensor_tensor(rsq[:], rsq[:], nt1[:], ALU.mult)

            # ---------------- unit-lower inverse, scale rows ----------------
            wu = chp.tile([P, D * D], F32, tag="wu")
            nc.vector.tensor_copy(wu[:], eye_k[:])
            wv = wu[:].rearrange("p (i c) -> p i c", c=D)
            for jc in range(D - 1):
                n = D - 1 - jc
                lcol = l_m[:, 32 * (jc + 1) + jc : D * D : 32]
                roww = wv[:, jc, 0 : jc + 1]
                tmpu = chtmp.tile([P, 31, 31], F32, tag="tmpu")
                nc.vector.tensor_tensor(
                    tmpu[:, 0:n, 0 : jc + 1],
                    lcol.unsqueeze(2).broadcast_to([P, n, jc + 1]),
                    roww.unsqueeze(1).broadcast_to([P, n, jc + 1]),
                    ALU.mult,
                )
                nc.vector.tensor_sub(
                    wv[:, jc + 1 : D, 0 : jc + 1],
                    wv[:, jc + 1 : D, 0 : jc + 1],
                    tmpu[:, 0:n, 0 : jc + 1],
                )
            nc.vector.tensor_tensor(
                wv, wv, rsq[:].unsqueeze(2).broadcast_to([P, D, D]), ALU.mult
            )

            # W^T (e-major) for the Wblk scatter; -W@new_mu for the bias
            wt = chp.tile([P, D * D], F32, tag="wt")
            nc.vector.tensor_copy(
                wt[:].rearrange("p (e d) -> p e d", d=D),
                wv.transpose([0, 2, 1]),
            )
            tmpw = chp.tile([P, D * D], F32, tag="tmpw")
            nc.vector.tensor_tensor(
                tmpw[:].rearrange("p (d e) -> p d e", e=D),
                wv,
                nmu[:].unsqueeze(1).broadcast_to([P, D, D]),
                ALU.mult,
            )
            wmu = small.tile([P, D], F32, tag="wmu")
            nc.vector.tensor_reduce(
                wmu[:], tmpw[:].rearrange("p (d e) -> p d e", e=D),
                mybir.AxisListType.X, ALU.add,
            )
            nc.vector.tensor_scalar_mul(wmu[:], wmu[:], -1.0)

            wtb = dr.tile([P, D * D], F32, tag="wtb")
            nc.sync.dma_start(wtb[:], wt[:])
            wmub = dr.tile([P, D], F32, tag="wmub")
            nc.sync.dma_start(wmub[:], wmu[:])
            for j in range(4):
                nc.sync.dma_start(
                    wblk[j:P:4, :].rearrange("e (g c) -> e g c", c=P)[
                        :, :, 32 * j : 32 * j + 32
                    ],
                    wtb[:].rearrange("(jj g) (e d) -> jj e g d", jj=4, d=D)[j],
                )
                nc.sync.dma_start(
                    wmur[:].rearrange("p (g c) -> p g c", c=P)[
                        :, :, 32 * j : 32 * j + 32
                    ],
                    wmub[:].rearrange("(jj g) d -> jj g d", jj=4)[j]
                    .unsqueeze(0).broadcast_to([P, 32, D]),
                )

            # ---------------- pass 2: whitening ----------------
            with (
                tc.tile_pool(name="xtp", bufs=2, space="PSUM") as xtps,
                tc.tile_pool(name="zp", bufs=2, space="PSUM") as zps,
            ):
                for t in range(NT):
                    xt2 = xpool.tile([P, DK], F32, tag="xt")
                    nc.sync.dma_start(xt2[:], xs[:][128 * t : 128 * (t + 1), :])
                    zt = zpool.tile([P, DK], F32, tag="zt")
                    for q in range(8):
                        pxt = xtps.tile([P, 512], F32, tag="pxt")
                        for gg in range(4):
                            g = 4 * q + gg
                            nc.tensor.transpose(
                                pxt[:, 128 * gg : 128 * (gg + 1)],
                                xt2[:, g:DK:32], idt[:],
                            )
                        xct = xctp.tile([P, 512], F32, tag="xct")
                        nc.scalar.copy(xct[:], pxt[:])
                        pz = zps.tile([P, 512], F32, tag="pz")
                        for gg in range(4):
                            g = 4 * q + gg
                            nc.tensor.matmul(
                                pz[:, 128 * gg : 128 * (gg + 1)],
                                xct[:, 128 * gg : 128 * (gg + 1)],
                                wblk[:, 128 * g : 128 * (g + 1)],
                                start=True, stop=True,
                            )
                        nc.vector.tensor_tensor(
                            zt[:].rearrange(
                                "p (d jj gb) -> p gb jj d", jj=4, gb=32
                            )[:, 4 * q : 4 * q + 4, :, :],
                            pz[:].rearrange("p (gg j d) -> p gg j d", gg=4, d=D),
                            wmur[:, 512 * q : 512 * (q + 1)].rearrange(
                                "p (gg j d) -> p gg j d", gg=4, d=D
                            ),
                            ALU.add,
                        )
                    nc.sync.dma_start(z_out[:][128 * t : 128 * (t + 1), :], zt[:])

    nc.compile()
    return nc


def _get_nc():
    if "nc" not in _CACHE:
        _CACHE["nc"] = _build()
    return _CACHE["nc"]


def kernel(x, mu_0, L_0, n_0):
    x = np.ascontiguousarray(np.asarray(x, dtype=np.float32))
    mu_0 = np.ascontiguousarray(np.asarray(mu_0, dtype=np.float32))
    L_0 = np.ascontiguousarray(np.asarray(L_0, dtype=np.float32))
    n_0 = np.ascontiguousarray(np.asarray(n_0, dtype=np.float32))

    nc = _get_nc()
    ident = np.eye(P, dtype=np.float32)
    eye_k = np.broadcast_to(
        np.eye(D, dtype=np.float32).reshape(1, D * D), (P, D * D)
    ).copy()
    onesb = np.ones((P, 1), dtype=ml_dtypes.bfloat16)

    x2 = x.reshape(B, DK)
    in_maps = []
    for c in range(N_CORES):
        in_maps.append({
            "xs": x2[c * BS : (c + 1) * BS],
            "mu0_in": mu_0,
            "l0_in": L_0.reshape(K, D * D),
            "n0_in": n_0,
            "ident_in": ident,
            "eye_in": eye_k,
            "ones_in": onesb,
        })
    res = run_bass_kernel_spmd(
        nc, in_maps, core_ids=list(range(N_CORES)),
        trace=bool(_CACHE.get("trace", False)),
    )
    _CACHE["last_res"] = res
    z = np.concatenate(
        [res.results[c]["z_out"] for c in range(N_CORES)], axis=0
    )
    return z.reshape(B, D, K)
